# revision 8
# baseline (speedup 1.0000x reference)
"""TRN2 Bass kernel: mention-span backbone (MLPs + S*S band joint + topk + BCE).

Strategy (8 NeuronCores, SPMD, fp32r matmuls):
  - Data-parallel over rows: core c handles example b=c//4, quarter q=c%4
    (1024 rows) with a 29-row halo on the end side.
  - Only the valid span band (start <= end < start+30) of the S*S logit matrix
    is materialized: [4096, 30] per example.
  - Both layernorms are folded through the big matmuls algebraically: heavy
    products run on raw gelu outputs; per-row/col LN fixups apply to the tiny
    [128,160] band tiles.
  - Device computes the exact 819/820 threshold per example (per-partition
    top-32 via max/match_replace + two-level gpsimd kth_largest after an
    AllGather of the band across each example's 4 cores).
  - Host extracts indices from band+threshold and re-ranks a small ambiguity
    window around the threshold in fp64 to absorb fp32r noise.
"""

import os
import numpy as np

import concourse.bass as bass
import concourse.tile as tile
from concourse import bacc, mybir
from concourse import bass_utils
from concourse.ap import AP

F32 = mybir.dt.float32
F32R = mybir.dt.float32r
U8 = mybir.dt.uint8
AF = mybir.ActivationFunctionType
OP = mybir.AluOpType

B, S, H, F = 2, 4096, 1024, 3072
MAX_SPAN = 30
TOP_LAMBDA = 0.2
LN_EPS = 1e-5
NCORE = 8
QS = S // 4              # 1024 rows per core
EC = QS + 32             # e-side cols incl. halo, padded
NT = QS // 128           # 8 s-tiles per core
WIN = 160                # band window width per s-tile
FC = F // 128            # 24
HC = H // 128            # 8
TOPK = int(S * TOP_LAMBDA)   # 819
NEG = -1.0e30
NBAND = sum(min(MAX_SPAN, S - s) for s in range(S))  # 122445 valid band slots
DELTA = 0.02             # host fix-up half-window vs device fp32r error

_prog_cache = {}
_bmask_cache = {}


def _r32(ap):
    return ap.bitcast(F32R)


def _build(consts):
    """Build the SPMD program. consts = (sw, swe, scs, swb, c_sl, c_el)."""
    sw, swe, scs, swb, c_sl, c_el = consts
    nc = bacc.Bacc("TRN2", target_bir_lowering=False, debug=False, num_devices=NCORE)

    seqT = nc.dram_tensor("seqT", [H, EC], F32R, kind="ExternalInput").ap()
    Wsr = nc.dram_tensor("Wsr", [FC, 128, HC, 128], F32R, kind="ExternalInput")
    Wer = nc.dram_tensor("Wer", [FC, 128, HC, 128], F32R, kind="ExternalInput")
    W2r = nc.dram_tensor("W2r", [FC, 128, FC, 128], F32R, kind="ExternalInput")
    fvecs = nc.dram_tensor("fvecs", [F, 8], F32, kind="ExternalInput").ap()
    packs_in = nc.dram_tensor("packs_in", [F, 5], F32R, kind="ExternalInput").ap()
    bmask = nc.dram_tensor("bmask", [NT, 128, WIN], F32, kind="ExternalInput").ap()

    band_out = nc.dram_tensor("band_out", [QS * MAX_SPAN], F32, kind="ExternalOutput")
    misc_out = nc.dram_tensor("misc_out", [1, 8], F32, kind="ExternalOutput").ap()

    he_d = nc.dram_tensor("he_d", [FC * 128 * EC], F32R)
    wth_d = nc.dram_tensor("wth_d", [FC * 128 * QS], F32R)
    scr_d = nc.dram_tensor("scr_d", [NT * 128 * WIN], F32)
    rows_d = nc.dram_tensor("rows_d", [4 * QS], F32)
    stats_d = nc.dram_tensor("stats_d", [7 * QS + 6 * EC], F32)
    bcomp = nc.dram_tensor("bcomp", [QS * MAX_SPAN], F32)
    gath = nc.dram_tensor("gath", [4 * QS * MAX_SPAN], F32)

    SB = [(0, 512), (512, 512)]              # s-blocks (own rows)
    EB = [(0, 512), (512, 512), (1024, 32)]  # e-blocks incl. halo pad

    with tile.TileContext(nc) as tc:
      with tc.tile_pool(name="const", bufs=1) as cpool, \
           tc.tile_pool(name="rows", bufs=1) as rpool:
        fv = cpool.tile([128, FC, 8], F32)
        nc.sync.dma_start(fv[:], fvecs.rearrange("(c p) k -> p c k", p=128))
        packs = cpool.tile([128, FC, 5], F32R)
        nc.sync.dma_start(packs[:], packs_in.rearrange("(c p) k -> p c k", p=128))
        onescol = packs[:, 0, 0:1]

        srow_s = rpool.tile([5, QS], F32)
        sq_s = rpool.tile([1, QS], F32)
        srow_e = rpool.tile([5, EC], F32)
        sq_e = rpool.tile([1, EC], F32)
        r1row = rpool.tile([1, QS], F32)
        epst = rpool.tile([1, 1], F32)
        nc.vector.memset(epst[:], LN_EPS)

        # ============ Phase 1+2: MLPs and WTh (hs resident) ================
        with tc.tile_pool(name="hsp", bufs=1) as hsp:
            hs = hsp.tile([128, FC, QS], F32R)

            with tc.tile_pool(name="seqp", bufs=1) as seqp:
                sq_t = seqp.tile([128, HC, EC], F32R)
                nc.sync.dma_start(sq_t[:], seqT.rearrange("(c p) s -> p c s", p=128))

                with tc.tile_pool(name="w", bufs=2) as wpool, \
                     tc.tile_pool(name="sqt", bufs=2) as sqpool, \
                     tc.tile_pool(name="mlpps", bufs=2, space="PSUM") as pspool:

                    # --- s-side MLP -> hs (resident) ---
                    with tc.tile_pool(name="stps_s", bufs=1, space="PSUM") as stpool:
                        sps = stpool.tile([5, QS], F32, tag="sps")
                        sqs = stpool.tile([1, QS], F32, tag="sqs")
                        for f in range(FC):
                            wt = wpool.tile([128, HC, 128], F32R, tag="w")
                            nc.sync.dma_start(wt[:], Wsr.ap()[f])
                            for (o, n) in SB:
                                ps = pspool.tile([128, 512], F32, tag="mlp")
                                for h in range(HC):
                                    nc.tensor.matmul(
                                        ps[:, :n], wt[:, h, :],
                                        sq_t[:, h, o:o + n],
                                        start=(h == 0), stop=(h == HC - 1))
                                nc.scalar.activation(
                                    hs[:, f, o:o + n], ps[:, :n], AF.Gelu,
                                    bias=fv[:, f, 4:5], scale=1.0)
                                nc.tensor.matmul(
                                    sps[:, o:o + n], packs[:, f, :],
                                    hs[:, f, o:o + n],
                                    start=(f == 0), stop=(f == FC - 1))
                                st = sqpool.tile([128, 512], F32R, tag="sq")
                                nc.vector.tensor_tensor(
                                    st[:, :n], hs[:, f, o:o + n].bitcast(F32),
                                    hs[:, f, o:o + n].bitcast(F32), OP.mult)
                                nc.tensor.matmul(
                                    sqs[:, o:o + n], onescol,
                                    st[:, :n],
                                    start=(f == 0), stop=(f == FC - 1))
                        nc.vector.tensor_copy(srow_s[:], sps[:])
                        nc.vector.tensor_copy(sq_s[:], sqs[:])

                    # --- e-side MLP -> he_d (streamed) ---
                    with tc.tile_pool(name="stps_e", bufs=1, space="PSUM") as stpool, \
                         tc.tile_pool(name="hep", bufs=2) as hepool:
                        spe = stpool.tile([5, EC], F32, tag="spe")
                        sqe = stpool.tile([1, EC], F32, tag="sqe")
                        for f in range(FC):
                            wt = wpool.tile([128, HC, 128], F32R, tag="w")
                            nc.sync.dma_start(wt[:], Wer.ap()[f])
                            het = hepool.tile([128, EC], F32R, tag="he")
                            for (o, n) in EB:
                                ps = pspool.tile([128, 512], F32, tag="mlp")
                                for h in range(HC):
                                    nc.tensor.matmul(
                                        ps[:, :n], wt[:, h, :],
                                        sq_t[:, h, o:o + n],
                                        start=(h == 0), stop=(h == HC - 1))
                                nc.scalar.activation(
                                    het[:, o:o + n], ps[:, :n], AF.Gelu,
                                    bias=fv[:, f, 5:6], scale=1.0)
                                nc.tensor.matmul(
                                    spe[:, o:o + n], packs[:, f, :],
                                    het[:, o:o + n],
                                    start=(f == 0), stop=(f == FC - 1))
                                st = sqpool.tile([128, 512], F32R, tag="sq")
                                nc.vector.tensor_tensor(
                                    st[:, :n], het[:, o:o + n].bitcast(F32),
                                    het[:, o:o + n].bitcast(F32), OP.mult)
                                nc.tensor.matmul(
                                    sqe[:, o:o + n], onescol,
                                    st[:, :n],
                                    start=(f == 0), stop=(f == FC - 1))
                            nc.sync.dma_start(
                                AP(tensor=he_d, offset=f * 128 * EC,
                                   ap=[[EC, 128], [1, EC]]),
                                het[:])
                        nc.vector.tensor_copy(srow_e[:], spe[:])
                        nc.vector.tensor_copy(sq_e[:], sqe[:])

            # --- Phase 2: WTh = W2'.T @ hs (streamed to DRAM) ---
            with tc.tile_pool(name="w2", bufs=2) as w2pool, \
                 tc.tile_pool(name="wtho", bufs=2) as wthopool, \
                 tc.tile_pool(name="wthps", bufs=2, space="PSUM") as wthps, \
                 tc.tile_pool(name="r1ps", bufs=1, space="PSUM") as r1ps:
                r1p = r1ps.tile([1, QS], F32)
                for fp in range(FC):
                    w2t = w2pool.tile([128, FC, 128], F32R, tag="w2")
                    nc.sync.dma_start(w2t[:], W2r.ap()[fp])
                    wo = wthopool.tile([128, QS], F32R, tag="wtho")
                    for (o, n) in SB:
                        ps = wthps.tile([128, 512], F32, tag="wth")
                        for f in range(FC):
                            nc.tensor.matmul(
                                ps[:, :n], w2t[:, f, :],
                                hs[:, f, o:o + n],
                                start=(f == 0), stop=(f == FC - 1))
                        nc.scalar.copy(wo[:, o:o + n], ps[:, :n])
                        nc.tensor.matmul(
                            r1p[:, o:o + n], onescol,
                            wo[:, o:o + n],
                            start=(fp == 0), stop=(fp == FC - 1))
                    nc.sync.dma_start(
                        AP(tensor=wth_d, offset=fp * 128 * QS,
                           ap=[[QS, 128], [1, QS]]),
                        wo[:])
                nc.vector.tensor_copy(r1row[:], r1p[:])

        # ============ Phase 3: derived per-row vectors =====================
        with tc.tile_pool(name="p34", bufs=1) as p34:
            # bounce stats rows through DRAM into partition-0 packed layout
            nc.sync.dma_start(
                AP(tensor=stats_d, offset=0, ap=[[QS, 5], [1, QS]]), srow_s[:])
            nc.sync.dma_start(
                AP(tensor=stats_d, offset=5 * QS, ap=[[1, QS]]), sq_s[:])
            nc.sync.dma_start(
                AP(tensor=stats_d, offset=6 * QS, ap=[[1, QS]]), r1row[:])
            nc.sync.dma_start(
                AP(tensor=stats_d, offset=7 * QS, ap=[[EC, 5], [1, EC]]), srow_e[:])
            nc.sync.dma_start(
                AP(tensor=stats_d, offset=7 * QS + 5 * EC, ap=[[1, EC]]), sq_e[:])
            rws = p34.tile([1, 7, QS], F32)    # sum_h, w.h, x, x, x, sq, r1
            nc.sync.dma_start(
                rws[:], AP(tensor=stats_d, offset=0, ap=[[7 * QS, 1], [QS, 7], [1, QS]]))
            rwe = p34.tile([1, 6, EC], F32)    # sum_h, x, we.h, cs.h, wb.h, sq
            nc.sync.dma_start(
                rwe[:], AP(tensor=stats_d, offset=7 * QS,
                           ap=[[6 * EC, 1], [EC, 6], [1, EC]]))

            drv4 = p34.tile([1, 4, QS], F32)    # r1, nrsig_s, alpha, sl
            drv = [drv4[0:1, r, :] for r in range(4)]
            erow4 = p34.tile([1, 4, EC], F32)   # mu_e, rsig_e, q, el_tot
            erow = [erow4[0:1, r, :] for r in range(4)]
            mu_s = p34.tile([1, QS], F32)
            ta = p34.tile([1, EC], F32)
            tb = p34.tile([1, EC], F32)
            tc_ = p34.tile([1, EC], F32)
            inv_f = 1.0 / float(F)

            # s side
            nc.vector.tensor_scalar_mul(mu_s[:], rws[0:1, 0, :], inv_f)
            nc.vector.tensor_scalar_mul(ta[:, :QS], rws[0:1, 5, :], inv_f)
            nc.vector.tensor_tensor(tb[:, :QS], mu_s[:], mu_s[:], OP.mult)
            nc.vector.tensor_sub(ta[:, :QS], ta[:, :QS], tb[:, :QS])       # var
            nc.scalar.activation(tb[:, :QS], ta[:, :QS], AF.Sqrt, bias=epst[:])
            nc.vector.reciprocal(ta[:, :QS], tb[:, :QS])                    # rsig
            nc.vector.tensor_scalar_mul(drv[1], ta[:, :QS], -1.0)           # nrsig
            nc.vector.tensor_tensor(drv[2], drv[1], mu_s[:], OP.mult)       # alpha
            nc.vector.tensor_scalar_mul(ta[:, :QS], mu_s[:], sw)
            nc.vector.tensor_sub(ta[:, :QS], ta[:, :QS], rws[0:1, 1, :])
            nc.vector.tensor_tensor(tb[:, :QS], ta[:, :QS], drv[1], OP.mult)
            nc.vector.tensor_scalar_add(drv[3], tb[:, :QS], c_sl)           # sl
            nc.vector.tensor_copy(drv[0], rws[0:1, 6, :])

            # e side
            nc.vector.tensor_scalar_mul(erow[0], rwe[0:1, 0, :], inv_f)
            nc.vector.tensor_scalar_mul(ta[:], rwe[0:1, 5, :], inv_f)
            nc.vector.tensor_tensor(tb[:], erow[0], erow[0], OP.mult)
            nc.vector.tensor_sub(ta[:], ta[:], tb[:])
            nc.scalar.activation(tb[:], ta[:], AF.Sqrt, bias=epst[:])
            nc.vector.reciprocal(erow[1], tb[:])                            # rsig_e
            # q = rsig_e*(cs.h - mu_e*scs)
            nc.vector.tensor_scalar_mul(ta[:], erow[0], scs)
            nc.vector.tensor_sub(ta[:], rwe[0:1, 3, :], ta[:])
            nc.vector.tensor_tensor(erow[2], ta[:], erow[1], OP.mult)
            # el_tot = rsig_e*((we.h - mu_e*swe) + (wb.h - mu_e*swb)) + c_el
            nc.vector.tensor_scalar_mul(ta[:], erow[0], swe)
            nc.vector.tensor_sub(ta[:], rwe[0:1, 2, :], ta[:])
            nc.vector.tensor_scalar_mul(tb[:], erow[0], swb)
            nc.vector.tensor_sub(tb[:], rwe[0:1, 4, :], tb[:])
            nc.vector.tensor_add(ta[:], ta[:], tb[:])
            nc.vector.tensor_tensor(tc_[:], ta[:], erow[1], OP.mult)
            nc.vector.tensor_scalar_add(erow[3], tc_[:], c_el)

            # bounce s-side rows to per-partition layout [128, 4, NT]
            nc.sync.dma_start(
                AP(tensor=rows_d, offset=0, ap=[[4 * QS, 1], [1, 4 * QS]]), drv4[:])
            persv = p34.tile([128, 4, NT], F32)
            nc.sync.dma_start(
                persv[:], AP(tensor=rows_d, offset=0,
                             ap=[[1, 128], [QS, 4], [128, NT]]))

            # broadcast e rows across partitions -> [128, 4, EC]
            ebc = p34.tile([128, 4, EC], F32)
            for r in range(4):
                nc.gpsimd.partition_broadcast(ebc[:, r, :], erow4[0:1, r, :])

            # ============ Phase 4: band tiles + junk + compact out =========
            bm = p34.tile([128, NT, WIN], F32)
            nc.sync.dma_start(bm[:], bmask.rearrange("t p w -> p t w"))
            rs = p34.tile([128, NT], F32)
            with tc.tile_pool(name="bandin", bufs=2) as bip, \
                 tc.tile_pool(name="bandt", bufs=2) as btp, \
                 tc.tile_pool(name="bandps", bufs=2, space="PSUM") as bpsp:
                for t in range(NT):
                    w0 = t * 128
                    wtht = bip.tile([128, FC, 128], F32R, tag="wtht")
                    nc.sync.dma_start(
                        wtht[:], AP(tensor=wth_d, offset=w0,
                                    ap=[[QS, 128], [128 * QS, FC], [1, 128]]))
                    hew = bip.tile([128, FC, WIN], F32R, tag="hew")
                    nc.sync.dma_start(
                        hew[:], AP(tensor=he_d, offset=w0,
                                   ap=[[EC, 128], [128 * EC, FC], [1, WIN]]))
                    bps = bpsp.tile([128, WIN], F32, tag="bandps")
                    for fp in range(FC):
                        nc.tensor.matmul(bps[:], wtht[:, fp, :],
                                         hew[:, fp, :],
                                         start=(fp == 0), stop=(fp == FC - 1))
                    u = btp.tile([128, WIN], F32, tag="u")
                    # U = (mu_eB * r1) - P1
                    nc.vector.scalar_tensor_tensor(
                        u[:], ebc[:, 0, w0:w0 + WIN], persv[:, 0, t:t + 1],
                        bps[:], OP.mult, OP.subtract)
                    v = btp.tile([128, WIN], F32, tag="v")
                    # V = (U * nrsig_s) * rsig_eB
                    nc.vector.scalar_tensor_tensor(
                        v[:], u[:], persv[:, 1, t:t + 1],
                        ebc[:, 1, w0:w0 + WIN], OP.mult, OP.mult)
                    # W = (q_B * alpha) + V
                    nc.vector.scalar_tensor_tensor(
                        u[:], ebc[:, 2, w0:w0 + WIN], persv[:, 2, t:t + 1],
                        v[:], OP.mult, OP.add)
                    # X = (el_B + sl) + W
                    nc.vector.scalar_tensor_tensor(
                        v[:], ebc[:, 3, w0:w0 + WIN], persv[:, 3, t:t + 1],
                        u[:], OP.add, OP.add)
                    nc.vector.tensor_scalar(u[:], v[:], 1.0e4, -1.0e4,
                                            OP.min, OP.max)
                    vm = btp.tile([128, WIN], F32, tag="vm")
                    nc.vector.tensor_tensor(vm[:], u[:], bm[:, t, :], OP.add)
                    # junk: ln(1 + exp(vm)), accumulate row sums
                    ex = btp.tile([128, WIN], F32, tag="ex")
                    nc.scalar.activation(ex[:], vm[:], AF.Exp)
                    jt = btp.tile([128, WIN], F32, tag="jt")
                    nc.scalar.activation(jt[:], ex[:], AF.Ln, bias=1.0,
                                         accum_out=rs[:, t:t + 1])
                    # band values out: tile -> scratch, diagonal -> bcomp
                    nc.sync.dma_start(
                        AP(tensor=scr_d, offset=t * 128 * WIN,
                           ap=[[WIN, 128], [1, WIN]]),
                        vm[:])
                    dg = btp.tile([128, MAX_SPAN], F32, tag="dg")
                    nc.sync.dma_start(
                        dg[:], AP(tensor=scr_d, offset=t * 128 * WIN,
                                  ap=[[WIN + 1, 128], [1, MAX_SPAN]]))
                    nc.sync.dma_start(
                        AP(tensor=bcomp, offset=t * 128 * MAX_SPAN,
                           ap=[[MAX_SPAN, 128], [1, MAX_SPAN]]),
                        dg[:])

            nc.sync.dma_start(band_out.ap(), bcomp.ap())

            # junk total: cross-partition reduce of row sums
            misc = p34.tile([1, 8], F32)
            rst = p34.tile([128, 1], F32)
            nc.vector.tensor_reduce(rst[:], rs[:], mybir.AxisListType.X, OP.add)
            nc.gpsimd.tensor_reduce(misc[:, 0:1], rst[:], mybir.AxisListType.C, OP.add)

            # ============ Phase 5: AllGather + exact threshold =============
            nc.gpsimd.collective_compute(
                "AllGather", OP.bypass,
                replica_groups=[[0, 1, 2, 3], [4, 5, 6, 7]],
                ins=[bcomp.ap().opt()], outs=[gath.ap().opt()])

            with tc.tile_pool(name="topk", bufs=1) as tkp:
                NPL = 4 * QS * MAX_SPAN // 128   # 960
                work = tkp.tile([128, NPL], F32)
                nc.sync.dma_start(work[:],
                                  gath.ap().rearrange("(p x) -> p x", p=128))
                c32 = tkp.tile([128, 32], F32)
                for r in range(4):
                    nc.vector.max(c32[:, r * 8:(r + 1) * 8], work[:])
                    if r < 3:
                        nc.vector.match_replace(
                            work[:], c32[:, r * 8:(r + 1) * 8], work[:], NEG)
                k1 = tkp.tile([1, 2], F32)
                # boundary 410/411 of 4096: (1-q)*4095 = 409.5
                nc.gpsimd.kth_largest(k1[:], c32[:], n_per_lane=32, k=410,
                                      quantile=1.0 - 409.5 / 4095.0)
                t1b = tkp.tile([128, 1], F32)
                nc.gpsimd.partition_broadcast(t1b[:], k1[0:1, 0:1])
                pred = tkp.tile([128, 32], U8)
                nc.vector.tensor_scalar(pred[:], c32[:], t1b[:], None, OP.is_gt)
                negs = tkp.tile([128, 32], F32)
                nc.vector.memset(negs[:], NEG)
                cm = tkp.tile([128, 32], F32)
                nc.vector.select(cm[:], pred[:], negs[:], c32[:])
                k2 = tkp.tile([1, 2], F32)
                # n_valid = 3686; boundary 409/410: (1-q)*3685 = 408.5
                nc.gpsimd.kth_largest(k2[:], cm[:], n_per_lane=32, k=409,
                                      quantile=1.0 - 408.5 / 3685.0)
                nc.vector.tensor_copy(misc[:, 1:3], k2[:1, 0:2])
                nc.vector.tensor_copy(misc[:, 3:5], k1[:1, 0:2])
            nc.sync.dma_start(misc_out, misc[:])

    nc.compile()
    return nc


# ======================= host side =========================================

def _gelu64(x):
    from scipy.special import erf
    return x * 0.5 * (1.0 + erf(x / np.sqrt(2.0)))


def _prep_weights(Ws, We, W_s2e, gs, betas, w_start, w_end, bs, be, b_s2e,
                  b_start, b_end):
    W2 = (W_s2e * gs[:, None]).astype(np.float32)
    wb = (W_s2e.T @ betas + b_s2e).astype(np.float32)
    cs = W2.sum(axis=0).astype(np.float32)
    wsf = (w_start * gs).astype(np.float32)
    consts = (float(wsf.sum()), float(w_end.sum()), float(cs.sum()),
              float(wb.sum()),
              float(b_start + np.dot(w_start, betas)),
              float(b_end))
    Wsr = np.ascontiguousarray(Ws.reshape(HC, 128, FC, 128).transpose(2, 1, 0, 3))
    Wer = np.ascontiguousarray(We.reshape(HC, 128, FC, 128).transpose(2, 1, 0, 3))
    W2r = np.ascontiguousarray(W2.reshape(FC, 128, FC, 128).transpose(2, 1, 0, 3))
    fvecs = np.zeros((F, 8), np.float32)
    fvecs[:, 4] = bs
    fvecs[:, 5] = be
    packs = np.zeros((F, 5), np.float32)
    packs[:, 0] = 1.0
    packs[:, 1] = wsf
    packs[:, 2] = w_end
    packs[:, 3] = cs
    packs[:, 4] = wb
    return Wsr, Wer, W2r, fvecs, packs, consts


def _make_bmask(q):
    """Additive band mask for core quarter q: 0 in-band, NEG outside."""
    m = np.full((NT, 128, WIN), NEG, np.float32)
    for t in range(NT):
        for p in range(128):
            s_glob = q * QS + t * 128 + p
            for d in range(MAX_SPAN):
                j = p + d
                if j >= WIN or s_glob + d >= S:
                    break
                m[t, p, j] = 0.0
    return m


def _exact_pairs(pairs, b, seq, Ws, bs, gs, betas, We, be, ge, betae,
                 w_start, b_start, w_end, b_end, W_s2e, b_s2e):
    """fp64 exact clipped band logits for (s, e) pairs of example b."""
    if not pairs:
        return {}
    s_need = sorted({s for s, _ in pairs})
    e_need = sorted({e for _, e in pairs})

    def mlp_rows(rows, W, bias, g, beta):
        x = seq[b, rows].astype(np.float64)
        h = _gelu64(x @ W.astype(np.float64) + bias.astype(np.float64))
        mu = h.mean(-1, keepdims=True)
        var = h.var(-1, keepdims=True)
        return (h - mu) / np.sqrt(var + LN_EPS) * g.astype(np.float64) + \
            beta.astype(np.float64)

    sr = mlp_rows(s_need, Ws, bs, gs, betas)
    er = mlp_rows(e_need, We, be, ge, betae)
    tmp = sr @ W_s2e.astype(np.float64) + b_s2e.astype(np.float64)
    sl = sr @ w_start.astype(np.float64) + float(b_start)
    el = er @ w_end.astype(np.float64) + float(b_end)
    si = {s: i for i, s in enumerate(s_need)}
    ei = {e: i for i, e in enumerate(e_need)}
    return {(s, e): min(max(float(tmp[si[s]] @ er[ei[e]] + sl[si[s]] + el[ei[e]]),
                            -1e4), 1e4)
            for (s, e) in pairs}


def _host_fallback(**inputs):
    """Pure-numpy reference replica (safety net; slow but exact)."""
    f32 = np.float32
    seq = np.asarray(inputs["sequence_output"], f32)
    am = np.asarray(inputs["attention_mask"])
    gold = np.asarray(inputs["gold_mentions"]).astype(np.int64)
    gm = np.asarray(inputs["gold_mentions_mask"], f32)

    def mlp(x, W, b, g, beta):
        h = _gelu64(x.astype(np.float64) @ np.asarray(W, f32).astype(np.float64)
                    + np.asarray(b, np.float64))
        mu = h.mean(-1, keepdims=True)
        var = h.var(-1, keepdims=True)
        return ((h - mu) / np.sqrt(var + LN_EPS) * np.asarray(g, np.float64)
                + np.asarray(beta, np.float64))

    starts = np.zeros((B, TOPK), np.int32)
    ends = np.zeros((B, TOPK), np.int32)
    span_mask = np.zeros((B, TOPK), np.int32)
    cost = np.zeros((B,), f32)
    lens = am.sum(-1)
    sv = np.triu(np.ones((S, S), f32)) * np.tril(np.ones((S, S), f32), MAX_SPAN - 1)
    for b in range(B):
        sr = mlp(seq[b], inputs["Ws"], inputs["bs"], inputs["gs"], inputs["betas"])
        er = mlp(seq[b], inputs["We"], inputs["be"], inputs["ge"], inputs["betae"])
        sl = sr @ np.asarray(inputs["w_start"], np.float64) + float(inputs["b_start"])
        el = er @ np.asarray(inputs["w_end"], np.float64) + float(inputs["b_end"])
        tmp = sr @ np.asarray(inputs["W_s2e"], np.float64) + \
            np.asarray(inputs["b_s2e"], np.float64)
        ml = tmp @ er.T + sl[:, None] + el[None, :]
        ml = np.clip(ml + (1.0 - sv) * (-1e4), -1e4, 1e4)
        k = int(float(lens[b]) * TOP_LAMBDA)
        flat = ml.reshape(-1)
        top = np.argpartition(flat, -TOPK)[-TOPK:]
        top = top[np.argsort(-flat[top], kind="stable")]
        smask = (np.arange(TOPK) < k).astype(np.int64)
        idx = top * smask + (1 - smask) * (S * S - 1)
        idx = np.sort(idx)
        idx = np.where(idx == S * S - 1, 0, idx)
        starts[b] = (idx // S).astype(np.int32)
        ends[b] = (idx % S).astype(np.int32)
        span_mask[b] = smask.astype(np.int32)
        probs = 1.0 / (1.0 + np.exp(-ml))
        gs_, ge_ = gold[b, :, 0], gold[b, :, 1]
        gp = probs[gs_, ge_]
        has_gold = gold[b].sum() > 0
        with np.errstate(divide="ignore"):
            lp = np.clip(np.log(gp * gm[b]), -100, None)
        cg = (-(gm[b] * lp)).sum() / gm[b].sum() if has_gold else 0.0
        junk = probs.copy()
        if has_gold:
            junk[gs_, ge_] = 0.0
            junk[0, 0] = probs[0, 0]
        mm = sv * (am[b].astype(f32)[:, None] * am[b].astype(f32)[None, :])
        pj = junk * mm
        with np.errstate(divide="ignore"):
            l1p = np.clip(np.log1p(-pj), -100, None)
        cj = (-l1p).sum() / mm.sum()
        cost[b] = np.float32(cg + cj)
    return (starts, ends, span_mask, inputs["sequence_output"], cost)


def kernel(**inputs):
    seq = np.ascontiguousarray(np.asarray(inputs["sequence_output"], np.float32))
    am = np.asarray(inputs["attention_mask"])
    gold = np.asarray(inputs["gold_mentions"]).astype(np.int64)
    gmask = np.asarray(inputs["gold_mentions_mask"], np.float32)
    Ws = np.asarray(inputs["Ws"], np.float32)
    We = np.asarray(inputs["We"], np.float32)
    W_s2e = np.asarray(inputs["W_s2e"], np.float32)
    bs = np.asarray(inputs["bs"], np.float32)
    be = np.asarray(inputs["be"], np.float32)
    gs = np.asarray(inputs["gs"], np.float32)
    ge = np.asarray(inputs["ge"], np.float32)
    betas = np.asarray(inputs["betas"], np.float32)
    betae = np.asarray(inputs["betae"], np.float32)
    w_start = np.asarray(inputs["w_start"], np.float32)
    w_end = np.asarray(inputs["w_end"], np.float32)
    b_start = float(np.asarray(inputs["b_start"]))
    b_end = float(np.asarray(inputs["b_end"]))
    b_s2e = np.asarray(inputs["b_s2e"], np.float32)

    # Device path folds gs/betas into weights but assumes trivial ge/betae
    # and a full attention mask; anything else -> exact host path.
    if not (np.all(ge == 1.0) and np.all(betae == 0.0) and np.all(am == 1)):
        return _host_fallback(**inputs)

    Wsr, Wer, W2r, fvecs, packs_np, consts = _prep_weights(
        Ws, We, W_s2e, gs, betas, w_start, w_end, bs, be, b_s2e, b_start, b_end)

    if consts not in _prog_cache:
        _prog_cache[consts] = _build(consts)
    nc = _prog_cache[consts]

    in_maps = []
    for c in range(NCORE):
        b, q = divmod(c, 4)
        lo = q * QS
        hi = min(S, lo + EC)
        st = np.zeros((H, EC), np.float32)
        st[:, :hi - lo] = seq[b, lo:hi].T
        if q not in _bmask_cache:
            _bmask_cache[q] = _make_bmask(q)
        in_maps.append({
            "seqT": np.ascontiguousarray(st),
            "Wsr": Wsr, "Wer": Wer, "W2r": W2r,
            "fvecs": fvecs, "packs_in": packs_np, "bmask": _bmask_cache[q],
        })

    trace = bool(int(os.environ.get("KERNEL_TRACE", "0")))
    res = bass_utils.run_bass_kernel_spmd(
        nc, in_maps, core_ids=list(range(NCORE)), trace=trace)
    if trace and res.exec_time_ns is not None:
        print(f"HW exec time: {res.exec_time_ns} ns")
        kernel._last_exec_ns = res.exec_time_ns

    band = np.zeros((B, S, MAX_SPAN), np.float32)
    junk_partial = np.zeros((B,), np.float64)
    thr = np.zeros((B, 2), np.float64)
    for c in range(NCORE):
        b, q = divmod(c, 4)
        band[b, q * QS:(q + 1) * QS] = \
            res.results[c]["band_out"].reshape(QS, MAX_SPAN)
        junk_partial[b] += float(res.results[c]["misc_out"][0, 0])
        if q == 0:
            thr[b, 0] = float(res.results[c]["misc_out"][0, 1])  # mid(819,820)
            thr[b, 1] = float(res.results[c]["misc_out"][0, 2])  # 820th value

    starts = np.zeros((B, TOPK), np.int32)
    ends = np.zeros((B, TOPK), np.int32)
    span_mask = np.ones((B, TOPK), np.int32)
    cost = np.zeros((B,), np.float32)

    ref_args = dict(Ws=Ws, bs=bs, gs=gs, betas=betas, We=We, be=be, ge=ge,
                    betae=betae, w_start=w_start, b_start=b_start, w_end=w_end,
                    b_end=b_end, W_s2e=W_s2e, b_s2e=b_s2e)

    for b in range(B):
        t = thr[b, 0]
        fb = band[b]
        sel = fb > t
        nsel = int(sel.sum())
        if not (TOPK - 64 <= nsel <= TOPK + 64):
            return _host_fallback(**inputs)
        amb = np.abs(fb - t) <= DELTA
        sure = sel & ~amb
        n_sure = int(sure.sum())
        need = TOPK - n_sure
        amb_idx = np.argwhere(amb)
        pairs = [(int(s), int(s + d)) for s, d in amb_idx]
        if need < 0 or need > len(pairs):
            return _host_fallback(**inputs)
        exact = _exact_pairs(pairs, b, seq, **ref_args)
        order = sorted(range(len(pairs)), key=lambda i: -exact[pairs[i]])
        chosen = [pairs[i] for i in order[:need]]
        sidx = [int(s) * S + int(s + d) for s, d in np.argwhere(sure)]
        sidx += [s * S + e for s, e in chosen]
        idx = np.sort(np.asarray(sidx, np.int64))
        if len(idx) != TOPK:
            return _host_fallback(**inputs)
        idx = np.where(idx == S * S - 1, 0, idx)
        starts[b] = (idx // S).astype(np.int32)
        ends[b] = (idx % S).astype(np.int32)

        # costs
        gs_, ge_ = gold[b, :, 0], gold[b, :, 1]
        has_gold = gold[b].sum() > 0
        inband = (gs_ <= ge_) & (ge_ < gs_ + MAX_SPAN) & (ge_ < S)
        G = gold.shape[1]
        gold_terms = np.full((G,), 100.0, np.float64)
        for g in range(G):
            if inband[g]:
                v = float(band[b, gs_[g], ge_[g] - gs_[g]])
                gold_terms[g] = min(np.log1p(np.exp(-v)) if v > -30 else -v, 100.0)
        gold_terms = gold_terms * (gmask[b] > 0)
        denom = gmask[b].sum()
        cg = gold_terms.sum() / denom if (has_gold and denom > 0) else 0.0

        jsum = junk_partial[b]
        if has_gold:
            seen = set()
            for g in range(G):
                p_ = (int(gs_[g]), int(ge_[g]))
                if inband[g] and p_ != (0, 0) and p_ not in seen:
                    seen.add(p_)
                    v = float(band[b, p_[0], p_[1] - p_[0]])
                    jsum -= float(np.log1p(np.exp(v)) if v < 30 else v)
        cj = jsum / float(NBAND)
        cost[b] = np.float32(cg + cj)

    return (starts, ends, span_mask, inputs["sequence_output"],
            cost.astype(np.float32))
